# revision 21
# baseline (speedup 1.0000x reference)
"""GumbelSelector Trainium2 kernel.

Math: h = relu(s @ W1 + b1); lo = h @ W2 + b2  (2 classes)
  dec  = (argmax(lo) == 1)  ==  (z > 0)         where z = h @ (W2[:,1]-W2[:,0]) + (b2[1]-b2[0])
  prob = softmax(lo)[..., 1] ==  sigmoid(z)
  Per-row correction (LB=1): if a row of dec is all zero, activate argmax(rnoise).

Sharding: data-parallel over batch B=64 -> 8 cores x 8 rows. Weights replicated.
Host pre-transposes each core's s shard to [D=256, 32768] so the DMA loads are
fully coalesced and the contraction dim lands on SBUF partitions directly.

Structure (all matmuls float32r: 1 PE cycle/row at free>=256, vs 4 for fp32):
- software pipeline one tile deep: tile i does layer-1 (PE) + relu (ACT)
  while tile i-1 does layer-2 (PE) + sigmoid (ACT), so the in-order
  sequencers never stall on a same-tile round trip.
- prob accumulates in [1, N]-row chunks; per chunk (= one batch row) the
  DRAM store, dec = prob > 0.5 (DVE), row-max, and dec8-row flush all run
  hidden under the DMA-bound main loop. Only the row-correction fix-up
  (two [8, N] DVE ops + final dec store) remains after the last tile.
"""

import sys

if "/opt/trn_rl_repo" not in sys.path:
    sys.path.insert(0, "/opt/trn_rl_repo")

import numpy as np

import concourse.bass as bass
import concourse.mybir as mybir
import concourse.tile as tile
from concourse import bacc
from concourse.bass_utils import run_bass_kernel_spmd

B, N, D = 64, 4096, 256
HID = D // 2  # 128
NCORES = 8
BPC = B // NCORES          # batch rows per core
TOK = BPC * N              # 32768 tokens per core
SLAB = 2048                # tokens per DMA slab (1 MiB per 128-partition load)
TS = 1024                  # tokens per compute tile (2 PSUM banks)
NT = TOK // TS             # 32 compute tiles
F32 = mybir.dt.float32
F32R = mybir.dt.float32r   # 1 cycle/row on the PE (vs 4 for fp32) at free>=256

_NC = None


def _build_nc():
    nc = bacc.Bacc("TRN2", target_bir_lowering=False, debug=False)
    sT = nc.dram_tensor("sT", [D, TOK], F32R, kind="ExternalInput")
    rn = nc.dram_tensor("rn", [BPC, N], F32, kind="ExternalInput")
    w1 = nc.dram_tensor("w1", [D, HID], F32R, kind="ExternalInput")
    b1 = nc.dram_tensor("b1", [HID, 1], F32, kind="ExternalInput")
    w2d = nc.dram_tensor("w2d", [HID, 1], F32R, kind="ExternalInput")
    b2d = nc.dram_tensor("b2d", [1, 1], F32, kind="ExternalInput")
    dec = nc.dram_tensor("dec", [BPC, N], F32, kind="ExternalOutput")
    prob = nc.dram_tensor("prob", [BPC, N], F32, kind="ExternalOutput")

    AF = mybir.ActivationFunctionType
    ALU = mybir.AluOpType

    with tile.TileContext(nc) as tc:
        with (
            tc.tile_pool(name="consts", bufs=1) as consts,
            tc.tile_pool(name="io8", bufs=1) as io8,
            tc.tile_pool(name="sapool", bufs=5) as sapool,
            tc.tile_pool(name="sbpool", bufs=5) as sbpool,
            tc.tile_pool(name="hpool", bufs=3) as hpool,
            tc.tile_pool(name="ckpool", bufs=3) as ckpool,
            tc.tile_pool(name="phpool", bufs=2, space=bass.MemorySpace.PSUM) as phpool,
            tc.tile_pool(name="pzpool", bufs=2, space=bass.MemorySpace.PSUM) as pzpool,
        ):
            # tiny const loads go first (the sync HWDGE ring is FIFO: a big
            # slab load ahead of them would delay the first matmul by the
            # whole slab transfer)
            w1a = consts.tile([128, HID], F32R)
            nc.scalar.dma_start(w1a[:], w1[0:128, :])
            w1b = consts.tile([128, HID], F32R)
            nc.scalar.dma_start(w1b[:], w1[128:256, :])
            b1s = consts.tile([HID, 1], F32)
            nc.scalar.dma_start(b1s[:], b1[:])
            w2s = consts.tile([HID, 1], F32R)
            nc.scalar.dma_start(w2s[:], w2d[:])
            b2s = consts.tile([1, 1], F32)
            nc.scalar.dma_start(b2s[:], b2d[:])
            rns = io8.tile([BPC, N], F32)
            nc.scalar.dma_start(rns[:], rn[:])

            # per-row max of rnoise, computed up front (overlaps main loop)
            rmaxr = io8.tile([BPC, 1], F32)
            nc.vector.tensor_reduce(rmaxr[:], rns[:], mybir.AxisListType.X, ALU.max)

            # prob rows land here via SBUF->SBUF flushes (engines can only
            # address base partitions 0/32/64/96, so ACT can't write row c);
            # dec is derived from it in one bulk pass at the end
            pc8 = io8.tile([BPC, N], F32)

            state = {"prev": None, "chunk": None}

            def stage2():
                h, toff = state["prev"]
                pz = pzpool.tile([1, TS], F32)
                nc.tensor.matmul(pz[0:1, 0:512], w2s[:], h[:, 0:512],
                                 start=True, stop=True)
                nc.tensor.matmul(pz[0:1, 512:1024], w2s[:], h[:, 512:1024],
                                 start=True, stop=True)
                coff = toff % N
                if coff == 0:
                    state["chunk"] = ckpool.tile([1, N], F32, name="chunk")
                chunk = state["chunk"]
                nc.scalar.activation(chunk[0:1, coff : coff + TS], pz[0:1, :],
                                     AF.Sigmoid, bias=b2s[:])
                if coff + TS == N:
                    c = toff // N
                    # prob store rides the ACT ring (its data is ready the
                    # moment the sequencer reaches it — no stall); the
                    # DVE-dependent dec8 flush goes via the idle GpSimd
                    # (SWDGE) ring so its sem wait can't stall relu/sigmoid
                    # or the slab loads (in-order sequencers)
                    nc.scalar.dma_start(prob[c : c + 1, :], chunk[:])
                    nc.gpsimd.dma_start(pc8[c : c + 1, :], chunk[:])

            for si in range(TOK // SLAB):
                off = si * SLAB
                sa = sapool.tile([128, SLAB], F32R)
                sb = sbpool.tile([128, SLAB], F32R)
                if si == 0:
                    # split the first slab into quarter loads so the first
                    # matmul's operand lands ~4x sooner (warmup)
                    q = SLAB // 4
                    for k in range(4):
                        nc.sync.dma_start(sa[:, k * q : (k + 1) * q],
                                          sT[0:128, off + k * q : off + (k + 1) * q])
                    for k in range(4):
                        nc.sync.dma_start(sb[:, k * q : (k + 1) * q],
                                          sT[128:256, off + k * q : off + (k + 1) * q])
                else:
                    nc.sync.dma_start(sa[:], sT[0:128, off : off + SLAB])
                    nc.sync.dma_start(sb[:], sT[128:256, off : off + SLAB])
                for part in range(SLAB // TS):
                    toff = si * SLAB + part * TS
                    hoff = part * TS
                    # layer-2 of the previous tile first: its deps are older,
                    # so the in-order PE/ACT sequencers never stall on a
                    # same-tile round-trip
                    if state["prev"] is not None:
                        stage2()
                    ph = phpool.tile([128, TS], F32)
                    nc.tensor.matmul(ph[:, 0:512], w1a[:],
                                     sa[:, hoff : hoff + 512],
                                     start=True, stop=False)
                    nc.tensor.matmul(ph[:, 512:1024], w1a[:],
                                     sa[:, hoff + 512 : hoff + 1024],
                                     start=True, stop=False)
                    nc.tensor.matmul(ph[:, 0:512], w1b[:],
                                     sb[:, hoff : hoff + 512],
                                     start=False, stop=True)
                    nc.tensor.matmul(ph[:, 512:1024], w1b[:],
                                     sb[:, hoff + 512 : hoff + 1024],
                                     start=False, stop=True)
                    h = hpool.tile([128, TS], F32R)
                    nc.scalar.activation(h[:], ph[:], AF.Relu, bias=b1s[:])
                    state["prev"] = (h, toff)
            stage2()

            # dec = (prob > 0.5) == (z > 0); rows with no active slot get
            # argmax(rnoise) forced on. pc8/rns are updated in place.
            nc.vector.tensor_scalar(pc8[:], pc8[:], 0.5, None, ALU.is_gt)
            rmaxd = io8.tile([BPC, 1], F32)
            nc.vector.tensor_reduce(rmaxd[:], pc8[:], mybir.AxisListType.X, ALU.max)
            need = io8.tile([BPC, 1], F32)
            nc.vector.tensor_scalar(need[:], rmaxd[:], 0.0, None, ALU.is_equal)
            nc.vector.tensor_scalar(rns[:], rns[:], rmaxr[:], need[:],
                                    ALU.is_equal, ALU.mult)
            nc.vector.tensor_max(pc8[:], pc8[:], rns[:])
            nc.sync.dma_start(dec[:], pc8[:])

    nc.compile()
    return nc


def _get_nc():
    global _NC
    if _NC is None:
        _NC = _build_nc()
    return _NC


def _make_in_maps(s, W1, b1, W2, b2, rnoise):
    s = np.ascontiguousarray(s, dtype=np.float32)
    w1 = np.ascontiguousarray(W1, dtype=np.float32)
    b1c = np.ascontiguousarray(b1, dtype=np.float32).reshape(HID, 1)
    w2dc = np.ascontiguousarray(W2[:, 1] - W2[:, 0], dtype=np.float32).reshape(HID, 1)
    b2dc = np.array([[b2[1] - b2[0]]], dtype=np.float32)
    rn = np.ascontiguousarray(rnoise, dtype=np.float32)

    # [NCORES, D, TOK] with the contraction dim outer -> coalesced loads
    sT = np.ascontiguousarray(
        s.reshape(NCORES, TOK, D).transpose(0, 2, 1)
    )
    return [
        {
            "sT": sT[c],
            "rn": rn.reshape(NCORES, BPC, N)[c],
            "w1": w1,
            "b1": b1c,
            "w2d": w2dc,
            "b2d": b2dc,
        }
        for c in range(NCORES)
    ]


def run(s, W1, b1, W2, b2, rnoise, trace=False):
    nc = _get_nc()
    in_maps = _make_in_maps(s, W1, b1, W2, b2, rnoise)
    res = run_bass_kernel_spmd(nc, in_maps, list(range(NCORES)), trace=trace)
    dec = np.concatenate([r["dec"] for r in res.results], axis=0)
    prob = np.concatenate([r["prob"] for r in res.results], axis=0)
    return (dec, prob), res


def kernel(s, W1, b1, W2, b2, rnoise):
    (dec, prob), _ = run(s, W1, b1, W2, b2, rnoise)
    return dec, prob


# revision 25
# speedup vs baseline: 1.0324x; 1.0324x over previous
"""GumbelSelector Trainium2 kernel.

Math: h = relu(s @ W1 + b1); lo = h @ W2 + b2  (2 classes)
  dec  = (argmax(lo) == 1)  ==  (z > 0)         where z = h @ (W2[:,1]-W2[:,0]) + (b2[1]-b2[0])
  prob = softmax(lo)[..., 1] ==  sigmoid(z)
  Per-row correction (LB=1): if a row of dec is all zero, activate argmax(rnoise).

Sharding: data-parallel over batch B=64 -> 8 cores x 8 rows. Weights replicated.
Host pre-transposes each core's s shard to [D=256, 32768] so the DMA loads are
fully coalesced and the contraction dim lands on SBUF partitions directly.

Structure (all matmuls float32r: 1 PE cycle/row at free>=256, vs 4 for fp32):
- software pipeline one tile deep: tile i does layer-1 (PE) + relu (ACT)
  while tile i-1 does layer-2 (PE) + sigmoid (ACT), so the in-order
  sequencers never stall on a same-tile round trip.
- prob accumulates in [1, N]-row chunks; per chunk (= one batch row) the
  DRAM store, dec = prob > 0.5 (DVE), row-max, and dec8-row flush all run
  hidden under the DMA-bound main loop. Only the row-correction fix-up
  (two [8, N] DVE ops + final dec store) remains after the last tile.
"""

import sys

if "/opt/trn_rl_repo" not in sys.path:
    sys.path.insert(0, "/opt/trn_rl_repo")

import numpy as np

import concourse.bass as bass
import concourse.mybir as mybir
import concourse.tile as tile
from concourse import bacc
from concourse.bass_utils import run_bass_kernel_spmd

B, N, D = 64, 4096, 256
HID = D // 2  # 128
NCORES = 8
BPC = B // NCORES          # batch rows per core
TOK = BPC * N              # 32768 tokens per core
SLAB = 2048                # tokens per DMA slab (1 MiB per 128-partition load)
TS = 1024                  # tokens per compute tile (2 PSUM banks)
NT = TOK // TS             # 32 compute tiles
F32 = mybir.dt.float32
F32R = mybir.dt.float32r   # 1 cycle/row on the PE (vs 4 for fp32) at free>=256

_NC = None


def _build_nc():
    nc = bacc.Bacc("TRN2", target_bir_lowering=False, debug=False)
    sT = nc.dram_tensor("sT", [D, TOK], F32R, kind="ExternalInput")
    rn = nc.dram_tensor("rn", [BPC, N], F32, kind="ExternalInput")
    w1 = nc.dram_tensor("w1", [D, HID], F32R, kind="ExternalInput")
    b1 = nc.dram_tensor("b1", [HID, 1], F32, kind="ExternalInput")
    w2d = nc.dram_tensor("w2d", [HID, 1], F32R, kind="ExternalInput")
    b2d = nc.dram_tensor("b2d", [1, 1], F32, kind="ExternalInput")
    dec = nc.dram_tensor("dec", [BPC, N], F32, kind="ExternalOutput")
    prob = nc.dram_tensor("prob", [BPC, N], F32, kind="ExternalOutput")

    AF = mybir.ActivationFunctionType
    ALU = mybir.AluOpType

    with tile.TileContext(nc) as tc:
        with (
            tc.tile_pool(name="consts", bufs=1) as consts,
            tc.tile_pool(name="io8", bufs=1) as io8,
            tc.tile_pool(name="sapool", bufs=5) as sapool,
            tc.tile_pool(name="sbpool", bufs=5) as sbpool,
            tc.tile_pool(name="hpool", bufs=4) as hpool,
            tc.tile_pool(name="ckpool", bufs=3) as ckpool,
            tc.tile_pool(name="phpool", bufs=1, space=bass.MemorySpace.PSUM) as phpool,
            tc.tile_pool(name="pzpool", bufs=1, space=bass.MemorySpace.PSUM) as pzpool,
        ):
            # tiny const loads go first (the sync HWDGE ring is FIFO: a big
            # slab load ahead of them would delay the first matmul by the
            # whole slab transfer)
            w1a = consts.tile([128, HID], F32R)
            nc.scalar.dma_start(w1a[:], w1[0:128, :])
            w1b = consts.tile([128, HID], F32R)
            nc.scalar.dma_start(w1b[:], w1[128:256, :])
            b1s = consts.tile([HID, 1], F32)
            nc.scalar.dma_start(b1s[:], b1[:])
            w2s = consts.tile([HID, 1], F32R)
            nc.scalar.dma_start(w2s[:], w2d[:])
            b2s = consts.tile([1, 1], F32)
            nc.scalar.dma_start(b2s[:], b2d[:])
            rns = io8.tile([BPC, N], F32)
            nc.scalar.dma_start(rns[:], rn[:])

            # per-row max of rnoise, computed up front (overlaps main loop)
            rmaxr = io8.tile([BPC, 1], F32)
            nc.vector.tensor_reduce(rmaxr[:], rns[:], mybir.AxisListType.X, ALU.max)

            # prob rows land here via SBUF->SBUF flushes (engines can only
            # address base partitions 0/32/64/96, so ACT can't write row c);
            # dec is derived from it in one bulk pass at the end
            pc8 = io8.tile([BPC, N], F32)

            # m8r[0, c] = rowmax(prob row c), written per chunk on partition 0
            # so the end-of-kernel fix-up can be branch-skipped cheaply
            m8r = io8.tile([1, BPC], F32)

            state = {"pgroup": None, "chunk": None}

            def stage2_pair():
                # layer-2 + sigmoid for both tiles of the previous slab,
                # grouped so the w2s stationary loads once per slab
                (hA, tA), (hB, tB) = state["pgroup"]
                pzA = pzpool.tile([1, TS], F32, name="pzA")
                pzB = pzpool.tile([1, TS], F32, name="pzB")
                nc.tensor.matmul(pzA[0:1, 0:512], w2s[:], hA[:, 0:512],
                                 start=True, stop=True)
                nc.tensor.matmul(pzA[0:1, 512:1024], w2s[:], hA[:, 512:1024],
                                 start=True, stop=True)
                nc.tensor.matmul(pzB[0:1, 0:512], w2s[:], hB[:, 0:512],
                                 start=True, stop=True)
                nc.tensor.matmul(pzB[0:1, 512:1024], w2s[:], hB[:, 512:1024],
                                 start=True, stop=True)
                for pz, toff in ((pzA, tA), (pzB, tB)):
                    coff = toff % N
                    if coff == 0:
                        state["chunk"] = ckpool.tile([1, N], F32, name="chunk")
                    chunk = state["chunk"]
                    nc.scalar.activation(chunk[0:1, coff : coff + TS],
                                         pz[0:1, :], AF.Sigmoid, bias=b2s[:])
                    if coff + TS == N:
                        c = toff // N
                        # prob store rides the ACT ring (data ready the moment
                        # the sequencer reaches it — no stall); pc8 flush goes
                        # via the idle GpSimd (SWDGE) ring so no in-order
                        # sequencer ever waits on a store
                        nc.scalar.dma_start(prob[c : c + 1, :], chunk[:])
                        nc.gpsimd.dma_start(pc8[c : c + 1, :], chunk[:])
                        nc.vector.tensor_reduce(m8r[0:1, c : c + 1], chunk[:],
                                                mybir.AxisListType.X, ALU.max)

            for si in range(TOK // SLAB):
                off = si * SLAB
                sa = sapool.tile([128, SLAB], F32R)
                sb = sbpool.tile([128, SLAB], F32R)
                if si == 0:
                    # split the first slab into quarter loads so the first
                    # matmul's operand lands ~4x sooner (warmup)
                    q = SLAB // 4
                    for k in range(4):
                        nc.sync.dma_start(sa[:, k * q : (k + 1) * q],
                                          sT[0:128, off + k * q : off + (k + 1) * q])
                    for k in range(4):
                        nc.sync.dma_start(sb[:, k * q : (k + 1) * q],
                                          sT[128:256, off + k * q : off + (k + 1) * q])
                else:
                    nc.sync.dma_start(sa[:], sT[0:128, off : off + SLAB])
                    nc.sync.dma_start(sb[:], sT[128:256, off : off + SLAB])
                # layer-2 of the previous slab first: its deps are older, so
                # the in-order PE/ACT sequencers never stall on a same-slab
                # round-trip
                if state["pgroup"] is not None:
                    stage2_pair()
                # layer-1 for both tiles of this slab, each stationary loaded
                # once: [w1a x4][w1b x4]
                phA = phpool.tile([128, TS], F32, name="phA")
                phB = phpool.tile([128, TS], F32, name="phB")
                nc.tensor.matmul(phA[:, 0:512], w1a[:], sa[:, 0:512],
                                 start=True, stop=False)
                nc.tensor.matmul(phA[:, 512:1024], w1a[:], sa[:, 512:1024],
                                 start=True, stop=False)
                nc.tensor.matmul(phB[:, 0:512], w1a[:], sa[:, 1024:1536],
                                 start=True, stop=False)
                nc.tensor.matmul(phB[:, 512:1024], w1a[:], sa[:, 1536:2048],
                                 start=True, stop=False)
                nc.tensor.matmul(phA[:, 0:512], w1b[:], sb[:, 0:512],
                                 start=False, stop=True)
                nc.tensor.matmul(phA[:, 512:1024], w1b[:], sb[:, 512:1024],
                                 start=False, stop=True)
                nc.tensor.matmul(phB[:, 0:512], w1b[:], sb[:, 1024:1536],
                                 start=False, stop=True)
                nc.tensor.matmul(phB[:, 512:1024], w1b[:], sb[:, 1536:2048],
                                 start=False, stop=True)
                hA = hpool.tile([128, TS], F32R, name="hA")
                nc.scalar.activation(hA[:], phA[:], AF.Relu, bias=b1s[:])
                hB = hpool.tile([128, TS], F32R, name="hB")
                nc.scalar.activation(hB[:], phB[:], AF.Relu, bias=b1s[:])
                state["pgroup"] = ((hA, off), (hB, off + TS))
            stage2_pair()

            # dec = (prob > 0.5) == (z > 0)
            nc.vector.tensor_scalar(pc8[:], pc8[:], 0.5, None, ALU.is_gt)
            # fix-up runs only if some row is all-inactive (rowmax prob <=
            # 0.5). Positive fp32 bit patterns order like the floats, so the
            # raw-bits register compare against 0.5f is exact.
            mmin = io8.tile([1, 1], F32)
            nc.vector.tensor_reduce(mmin[:], m8r[:], mybir.AxisListType.X,
                                    ALU.min)
            v = nc.vector.value_load(mmin[0:1, 0:1].bitcast(mybir.dt.int32))
            with tc.If(v <= 0x3F000000):
                rmaxd = io8.tile([BPC, 1], F32)
                nc.vector.tensor_reduce(rmaxd[:], pc8[:],
                                        mybir.AxisListType.X, ALU.max)
                need = io8.tile([BPC, 1], F32)
                nc.vector.tensor_scalar(need[:], rmaxd[:], 0.0, None,
                                        ALU.is_equal)
                nc.vector.tensor_scalar(rns[:], rns[:], rmaxr[:], need[:],
                                        ALU.is_equal, ALU.mult)
                nc.vector.tensor_max(pc8[:], pc8[:], rns[:])
            nc.sync.dma_start(dec[:], pc8[:])

    nc.compile()
    return nc


def _get_nc():
    global _NC
    if _NC is None:
        _NC = _build_nc()
    return _NC


def _make_in_maps(s, W1, b1, W2, b2, rnoise):
    s = np.ascontiguousarray(s, dtype=np.float32)
    w1 = np.ascontiguousarray(W1, dtype=np.float32)
    b1c = np.ascontiguousarray(b1, dtype=np.float32).reshape(HID, 1)
    w2dc = np.ascontiguousarray(W2[:, 1] - W2[:, 0], dtype=np.float32).reshape(HID, 1)
    b2dc = np.array([[b2[1] - b2[0]]], dtype=np.float32)
    rn = np.ascontiguousarray(rnoise, dtype=np.float32)

    # [NCORES, D, TOK] with the contraction dim outer -> coalesced loads
    sT = np.ascontiguousarray(
        s.reshape(NCORES, TOK, D).transpose(0, 2, 1)
    )
    return [
        {
            "sT": sT[c],
            "rn": rn.reshape(NCORES, BPC, N)[c],
            "w1": w1,
            "b1": b1c,
            "w2d": w2dc,
            "b2d": b2dc,
        }
        for c in range(NCORES)
    ]


def run(s, W1, b1, W2, b2, rnoise, trace=False):
    nc = _get_nc()
    in_maps = _make_in_maps(s, W1, b1, W2, b2, rnoise)
    res = run_bass_kernel_spmd(nc, in_maps, list(range(NCORES)), trace=trace)
    dec = np.concatenate([r["dec"] for r in res.results], axis=0)
    prob = np.concatenate([r["prob"] for r in res.results], axis=0)
    return (dec, prob), res


def kernel(s, W1, b1, W2, b2, rnoise):
    (dec, prob), _ = run(s, W1, b1, W2, b2, rnoise)
    return dec, prob
